# revision 11
# baseline (speedup 1.0000x reference)
"""Trainium2 Bass kernel for nn_Attention_75814762709205.

Computation (per batch row b, seq s):
    proj  = hidden_enc @ W + b          [B,S,D]
    score = hidden_dec.T * proj         (per-channel scale)
    attn  = softmax(score, axis=-1)     (over D)
    out   = sum_s attn * hidden_enc     [B,D]

Sharding: data-parallel over batch, 4 batches per core on 8 cores.

Precision strategy (validated numerically against the exact reference
data): the softmax error from an fp8 matmul is amplified by exp() in
proportion to |dec_e|, so the channels are permuted by |dec| on the
host.  The top NB=256 channels go through a bf16 matmul; the remaining
NF=768 channels go through an fp8e4 DoubleRow matmul.  dec is folded
into W on the host (score = enc @ (W*dec)).

V4 data-stream layout: the attention VALUES arrive as one fp8 byte
stream encv = [enc8 | er8_hi] (enc8 = fp8(enc); er8 = fp8(enc - enc8)
for the top NB cols).  The top-NB bf16 values are reconstructed on DVE
as enc8 + er8 (0.2% error, better than bf16); the low-channel values
are used in fp8 directly (they feed the x64-scaled fp8 ctx matmul).
This cuts the value stream from 16MB bf16 to 10MB fp8 and keeps one
DMA per stream-tile.  The bf16 transposed enc (matmul stationary) and
the fp8 SW-interleaved copy are host-prepared as before.

Softmax denominator is folded into the final sequence reduction:
context = sum_s r_s * (exp_s * enc_s) with r = 1/sum(exp).  The ctx
matmuls run two tiles behind so the PE never waits on the ACT/DVE
softmax tail.  PSUM: 2 pr bufs (2 banks each) + 2 ctx bufs (2 banks
each) so consecutive batches' ctx accumulators don't serialize.
"""

import sys

sys.path.insert(0, "/opt/trn_rl_repo")

from collections import deque

import numpy as np
import ml_dtypes

import concourse.bass as bass
import concourse.mybir as mybir
import concourse.tile as tile
from concourse import bacc, bass_utils

B, S, D = 32, 2048, 1024
NCORES = 8
BPC = B // NCORES  # batches per core
ROWS = BPC * S  # rows per core
P = 128
NT = ROWS // P  # row tiles per core
TPB = S // P  # row tiles per batch
KC = D // P  # contraction chunks
NB = 128  # bf16 (high |dec|) channels
NF = D - NB  # fp8 channels
W8SCALE = 8.0  # fp8 W pre-scale (undone in the exp activation)
WT8SCALE = 64.0  # fp8 wt pre-scale (keeps softmax weights in normal range;
                 # undone on the host after the output gather)

F32 = mybir.dt.float32
BF16 = mybir.dt.bfloat16
F8 = mybir.dt.float8e4
AF = mybir.ActivationFunctionType
DRS = mybir.MatmulPerfMode.DoubleRowSwInterleave
MUL = mybir.AluOpType.mult
ADD = mybir.AluOpType.add

NP_BF16 = ml_dtypes.bfloat16
NP_F8 = ml_dtypes.float8_e4m3

ENCV = D + NB  # merged fp8 value stream width


def build_program(with_bias: bool, repeats: int = 1):
    nc = bacc.Bacc("TRN2", target_bir_lowering=False, debug=False)
    encv_d = nc.dram_tensor("encv", [ROWS, ENCV], F8, kind="ExternalInput")
    encT_d = nc.dram_tensor("encT", [NT, P, KC, P], BF16, kind="ExternalInput")
    encT8_d = nc.dram_tensor("encT8", [NT, P, KC // 2, 2 * P], F8, kind="ExternalInput")
    wb_d = nc.dram_tensor("wb", [P, KC, NB], BF16, kind="ExternalInput")
    w8_d = nc.dram_tensor("w8", [P, KC, NF], F8, kind="ExternalInput")
    bb_d = b8_d = None
    if with_bias:
        bb_d = nc.dram_tensor("bb", [1, NB], BF16, kind="ExternalInput")
        b8_d = nc.dram_tensor("b8", [1, NF], BF16, kind="ExternalInput")
    out_d = nc.dram_tensor("out", [BPC, D], F32, kind="ExternalOutput")

    NTOT = repeats * NT

    with tile.TileContext(nc) as tc:
        with (
            tc.tile_pool(name="consts", bufs=1) as consts,
            tc.tile_pool(name="wpool", bufs=1) as wpool,
            tc.tile_pool(name="encvp", bufs=8) as encvp,
            tc.tile_pool(name="encTp", bufs=7) as encTp,
            tc.tile_pool(name="encT8p", bufs=7) as encT8p,
            tc.tile_pool(name="valhip", bufs=3) as valhip,
            tc.tile_pool(name="expp", bufs=3) as expp,
            tc.tile_pool(name="wtp", bufs=3) as wtp,
            tc.tile_pool(name="wt8p", bufs=3) as wt8p,
            tc.tile_pool(name="smalls", bufs=12) as smalls,
            tc.tile_pool(name="outp", bufs=2) as outp,
            tc.tile_pool(name="pr_ps", bufs=2, space=bass.MemorySpace.PSUM) as pr_ps,
            tc.tile_pool(name="ctx_ps", bufs=2, space=bass.MemorySpace.PSUM) as ctx_ps,
        ):
            # three DMA streams per tile, issue spread over SP, ScalarE and
            # Pool queues to avoid serializing on one descriptor queue
            loaded = {}

            def load(t):
                encv_t = encvp.tile([P, ENCV], F8, name="encv_t")
                nc.sync.dma_start(
                    encv_t, encv_d.ap()[(t % NT) * P : (t % NT + 1) * P, :]
                )
                encT = encTp.tile([P, KC, P], BF16, name="encT")
                nc.gpsimd.dma_start(encT, encT_d.ap()[t % NT])
                encT8 = encT8p.tile([P, KC // 2, 2 * P], F8, name="encT8")
                nc.scalar.dma_start(encT8, encT8_d.ap()[t % NT])
                loaded[t] = (encv_t, encT, encT8)

            state = {"ctx_half": None, "ones_col": None, "ones8": None}
            NPAIR = TPB // 2

            def emit_ctx(prev):
                # ctx for a PAIR of row tiles: the high-|dec| NB columns via
                # two bf16 matmuls (one per tile, out partition 0); the low
                # 768 via fp8 DoubleRowSwInterleave contracting both tiles'
                # 256 rows per pass.  The fp8 ones stationary is padded to
                # M=128 (full 256 active cols) -> 128 identical output
                # rows; row 0 is read.
                wt_hi2, wt82, pib, bidx = prev
                if pib == 0:
                    state["ctx_half"] = [
                        ctx_ps.tile([P, 512], F32, name=f"ctxh{h}") for h in range(2)
                    ]
                ctxh0, ctxh1 = state["ctx_half"]
                last = pib == NPAIR - 1
                for i in range(2):
                    nc.tensor.matmul(
                        ctxh0[0:1, 0:NB],
                        state["ones_col"],
                        wt_hi2[:, i, :],
                        start=(pib == 0 and i == 0),
                        stop=False,
                        skip_group_check=True,
                    )
                nc.tensor.matmul(
                    ctxh0[:, NB:512],
                    state["ones8"],
                    wt82[:, :, 0 : 512 - NB],
                    start=False,
                    stop=last,
                    perf_mode=DRS,
                    skip_group_check=True,
                )
                nc.tensor.matmul(
                    ctxh1,
                    state["ones8"],
                    wt82[:, :, 512 - NB : NF],
                    start=(pib == 0),
                    stop=last,
                    perf_mode=DRS,
                    skip_group_check=True,
                )
                if last:
                    # the two PSUM->SBUF drains run on DVE and ACT in
                    # parallel (both are [1,512] single-lane copies)
                    ctx_sb = outp.tile([1, D], F32, name="ctx_sb")
                    nc.vector.tensor_copy(
                        ctx_sb[:, 0:512], state["ctx_half"][0][0:1, :]
                    )
                    nc.scalar.add(
                        ctx_sb[:, 512:1024], state["ctx_half"][1][0:1, :], 0.0
                    )
                    nc.sync.dma_start(out_d.ap()[bidx : bidx + 1, :], ctx_sb)

            # load(0) first so tile 0's streams lead their queues; the W
            # DMAs follow split across the SP and ScalarE queues (the fp8
            # half used by the first two k-pairs rides ScalarE right after
            # encT8(0)) so the first-tile matmuls start ~2us in
            load(0)
            wb_sb = wpool.tile([P, KC, NB], BF16)
            nc.sync.dma_start(wb_sb, wb_d.ap())
            w8_sb = wpool.tile([P, KC, NF], F8)
            nc.scalar.dma_start(w8_sb[:, 0 : KC // 2, :], w8_d.ap()[:, 0 : KC // 2, :])
            nc.sync.dma_start(
                w8_sb[:, KC // 2 : KC, :], w8_d.ap()[:, KC // 2 : KC, :]
            )
            load(1)
            load(2)
            load(3)
            load(4)

            # constant ones stationaries for the ctx (sequence-sum) matmuls;
            # the softmax reciprocal is folded into wt on DVE instead
            ones_f = consts.tile([P, 1], F32)
            nc.any.memset(ones_f, 1.0)
            ones_col = consts.tile([P, 1], BF16)
            nc.vector.tensor_copy(ones_col, ones_f)
            state["ones_col"] = ones_col
            ones8_f = consts.tile([P, 2 * P], F32)
            nc.any.memset(ones8_f, 1.0)
            ones8 = consts.tile([P, 2 * P], F8)
            nc.vector.tensor_copy(ones8, ones8_f)
            state["ones8"] = ones8

            ones_row = None
            bb_sb = b8_sb = None
            if with_bias:
                ones_f32 = consts.tile([1, P], F32)
                nc.any.memset(ones_f32, 1.0)
                ones_row = consts.tile([1, P], BF16)
                nc.vector.tensor_copy(ones_row, ones_f32)
                bb_sb = consts.tile([1, NB], BF16)
                nc.sync.dma_start(bb_sb, bb_d.ap())
                b8_sb = consts.tile([1, NF], BF16)
                nc.sync.dma_start(b8_sb, b8_d.ap())

            pending = deque()
            for t in range(NTOT):
                bidx, tib = divmod(t % NT, TPB)
                if t + 5 < NTOT:
                    load(t + 5)
                encv_t, encT, encT8 = loaded.pop(t)

                # score layout (permuted channels): cols 0:NB bf16 part,
                # NB:1024 fp8 part.  One 2-bank PSUM tile; bank0 holds
                # [bf16 256 | fp8 256], bank1 holds fp8 512.
                pr = pr_ps.tile([P, 1024], F32, name="pr")

                # bf16 part: shares bank0 with the first fp8 region
                # (disjoint columns; the fp8 matmuls use start=False and
                # overwrite-on-first-touch via has_written)
                for k in range(KC):
                    nc.tensor.matmul(
                        pr[:, 0:NB],
                        encT[:, k, :],
                        wb_sb[:, k, :],
                        start=(k == 0),
                        stop=False,
                        skip_group_check=True,
                    )
                # fp8 DoubleRow part (software-interleaved stationary: the
                # host lays each k-pair out contiguously, so LDWEIGHTS is a
                # linear read instead of the HW interleave pattern)
                for c in range(KC // 2):
                    if c == 2 and len(pending) >= 2:
                        emit_ctx(pending.popleft())
                    # near the end of the program drain the ctx backlog
                    # eagerly so the post-loop tail stays short
                    if c == 0 and t + 4 >= NTOT and pending:
                        emit_ctx(pending.popleft())
                    ks = slice(2 * c, 2 * c + 2)
                    last = c == KC // 2 - 1
                    nc.tensor.matmul(
                        pr[:, NB:512],
                        encT8[:, c, :],
                        w8_sb[:, ks, 0 : 512 - NB],
                        start=False,
                        stop=(last and not with_bias),
                        perf_mode=DRS,
                        skip_group_check=True,
                    )
                    nc.tensor.matmul(
                        pr[:, 512:1024],
                        encT8[:, c, :],
                        w8_sb[:, ks, 512 - NB : NF],
                        start=(c == 0),
                        stop=(last and not with_bias),
                        perf_mode=DRS,
                        skip_group_check=True,
                    )
                if with_bias:
                    nc.tensor.matmul(
                        pr[:, 0:NB],
                        ones_row,
                        bb_sb,
                        start=False,
                        stop=False,
                        skip_group_check=True,
                    )
                    nc.tensor.matmul(
                        pr[:, NB:512],
                        ones_row,
                        b8_sb[:, 0 : 512 - NB],
                        start=False,
                        stop=True,
                        skip_group_check=True,
                    )
                    nc.tensor.matmul(
                        pr[:, 512:1024],
                        ones_row,
                        b8_sb[:, 512 - NB : NF],
                        start=False,
                        stop=True,
                        skip_group_check=True,
                    )

                # single exp over both banks with fused row-sum; the whole W
                # carries the W8SCALE pre-scale (exact power of 2), undone here
                ssum = smalls.tile([P, 1], F32)
                exp_t = expp.tile([P, D], BF16)
                nc.scalar.activation(
                    exp_t, pr, AF.Exp, scale=1.0 / W8SCALE, accum_out=ssum
                )

                recip_f = smalls.tile([P, 1], F32)
                nc.vector.reciprocal(recip_f, ssum)
                recip64 = smalls.tile([P, 1], F32)
                nc.vector.tensor_scalar_mul(recip64, recip_f, WT8SCALE)

                # reconstruct the bf16 high-|dec| values: enc8 + er8
                vals_hi = valhip.tile([P, NB], BF16, name="vals_hi")
                nc.vector.scalar_tensor_tensor(
                    vals_hi, encv_t[:, 0:NB], 1.0, encv_t[:, D:ENCV],
                    op0=MUL, op1=ADD,
                )

                # wt = (exp * 1/rowsum) * enc fused on DVE, written into
                # tile-PAIR buffers: bf16 for the high-|dec| block, fp8
                # (x64 pre-scaled) for the rest
                if t % 2 == 0:
                    state["wt_hi2"] = wtp.tile([P, 2, NB], BF16, name="wt_hi2")
                    state["wt82"] = wt8p.tile([P, 2, NF], F8, name="wt82")
                half = t % 2
                nc.vector.scalar_tensor_tensor(
                    state["wt_hi2"][:, half, :], exp_t[:, 0:NB], recip_f,
                    vals_hi,
                    op0=MUL, op1=MUL,
                )
                nc.vector.scalar_tensor_tensor(
                    state["wt82"][:, half, :], exp_t[:, NB:D], recip64,
                    encv_t[:, NB:D],
                    op0=MUL, op1=MUL,
                )

                if t % 2 == 1:
                    pending.append(
                        (state["wt_hi2"], state["wt82"], (tib // 2), bidx)
                    )
            while pending:
                emit_ctx(pending.popleft())

    nc.compile()
    return nc


def _perm(dec):
    return np.argsort(-np.abs(dec), kind="stable")


def make_in_maps(hidden_dec, hidden_enc, W, b):
    enc = np.asarray(hidden_enc, dtype=np.float32).reshape(B, S, D)
    W = np.asarray(W, dtype=np.float32)
    dec = np.asarray(hidden_dec, dtype=np.float32).reshape(D)
    b = np.asarray(b, dtype=np.float32).reshape(D)
    with_bias = bool(np.any(b != 0.0))

    perm = _perm(dec)
    Weff = W * dec[None, :]
    Wp = Weff[np.ix_(perm, perm)]
    wb = np.ascontiguousarray(
        (Wp[:, :NB] * W8SCALE).reshape(KC, P, NB).transpose(1, 0, 2)
    ).astype(NP_BF16)
    w8 = np.ascontiguousarray(
        (Wp[:, NB:] * W8SCALE).reshape(KC, P, NF).transpose(1, 0, 2)
    ).astype(NP_F8)
    encp = enc[:, :, perm].astype(NP_BF16)

    b_eff = (b * dec)[perm] * W8SCALE
    bb = b_eff[:NB].reshape(1, NB).astype(NP_BF16)
    b8 = b_eff[NB:].reshape(1, NF).astype(NP_BF16)

    in_maps = []
    for c in range(NCORES):
        ev = encp[c * BPC : (c + 1) * BPC].reshape(ROWS, D)
        ev8 = ev.astype(NP_F8)
        er8_hi = (
            ev[:, :NB].astype(np.float32) - ev8[:, :NB].astype(np.float32)
        ).astype(NP_F8)
        encv = np.concatenate([ev8, er8_hi], axis=1)
        # host-side tiled transpose into the exact SBUF stationary layout:
        # encT[t, p, kc, r] = enc[t*128 + r, kc*128 + p]
        encT = np.ascontiguousarray(
            ev.reshape(NT, P, KC, P).transpose(0, 3, 2, 1)
        )
        # fp8 copy, software-interleaved for DoubleRowSwInterleave: per
        # partition each k-pair's stationary stream is
        # [A_col127, B_col127, A_col126, ..., B_col0] (A/B = the two
        # k-chunks, columns reversed)
        e8rev = encT.astype(NP_F8)[:, :, :, ::-1]
        enc8i = np.ascontiguousarray(
            e8rev.reshape(NT, P, KC // 2, 2, P).transpose(0, 1, 2, 4, 3)
        ).reshape(NT, P, KC // 2, 2 * P)
        m = {
            "encv": np.ascontiguousarray(encv),
            "encT": encT,
            "encT8": enc8i,
            "wb": wb,
            "w8": w8,
        }
        if with_bias:
            m["bb"] = bb
            m["b8"] = b8
        in_maps.append(m)
    return in_maps, with_bias


def kernel(hidden_dec, hidden_enc, W, b):
    in_maps, with_bias = make_in_maps(hidden_dec, hidden_enc, W, b)
    nc = build_program(with_bias)
    res = bass_utils.run_bass_kernel_spmd(nc, in_maps, core_ids=list(range(NCORES)))
    outp = np.concatenate([res.results[c]["out"] for c in range(NCORES)], axis=0)
    outp[:, NB:] /= WT8SCALE  # undo the fp8 wt pre-scale on the low block
    perm = _perm(np.asarray(hidden_dec, dtype=np.float32).reshape(D))
    out = np.empty_like(outp)
    out[:, perm] = outp
    return out.astype(np.float32)


# revision 12
# speedup vs baseline: 1.0109x; 1.0109x over previous
"""Trainium2 Bass kernel for nn_Attention_75814762709205.

Computation (per batch row b, seq s):
    proj  = hidden_enc @ W + b          [B,S,D]
    score = hidden_dec.T * proj         (per-channel scale)
    attn  = softmax(score, axis=-1)     (over D)
    out   = sum_s attn * hidden_enc     [B,D]

Sharding: data-parallel over batch, 4 batches per core on 8 cores.

Precision strategy (validated numerically against the exact reference
data): the softmax error from an fp8 matmul is amplified by exp() in
proportion to |dec_e|, so the channels are permuted by |dec| on the
host.  The top NB=256 channels go through a bf16 matmul; the remaining
NF=768 channels go through an fp8e4 DoubleRow matmul.  dec is folded
into W on the host (score = enc @ (W*dec)).

V4 data-stream layout: the attention VALUES arrive as one fp8 byte
stream encv = [enc8 | er8_hi] (enc8 = fp8(enc); er8 = fp8(enc - enc8)
for the top NB cols).  The top-NB bf16 values are reconstructed on DVE
as enc8 + er8 (0.2% error, better than bf16); the low-channel values
are used in fp8 directly (they feed the x64-scaled fp8 ctx matmul).
This cuts the value stream from 16MB bf16 to 10MB fp8 and keeps one
DMA per stream-tile.  The bf16 transposed enc (matmul stationary) and
the fp8 SW-interleaved copy are host-prepared as before.

Softmax denominator is folded into the final sequence reduction:
context = sum_s r_s * (exp_s * enc_s) with r = 1/sum(exp).  The ctx
matmuls run two tiles behind so the PE never waits on the ACT/DVE
softmax tail.  PSUM: 2 pr bufs (2 banks each) + 2 ctx bufs (2 banks
each) so consecutive batches' ctx accumulators don't serialize.
"""

import sys

sys.path.insert(0, "/opt/trn_rl_repo")

from collections import deque

import numpy as np
import ml_dtypes

import concourse.bass as bass
import concourse.mybir as mybir
import concourse.tile as tile
from concourse import bacc, bass_utils

B, S, D = 32, 2048, 1024
NCORES = 8
BPC = B // NCORES  # batches per core
ROWS = BPC * S  # rows per core
P = 128
NT = ROWS // P  # row tiles per core
TPB = S // P  # row tiles per batch
KC = D // P  # contraction chunks
NB = 128  # bf16 (high |dec|) channels
NF = D - NB  # fp8 channels
W8SCALE = 8.0  # fp8 W pre-scale (undone in the exp activation)
WT8SCALE = 64.0  # fp8 wt pre-scale (keeps softmax weights in normal range;
                 # undone on the host after the output gather)

F32 = mybir.dt.float32
BF16 = mybir.dt.bfloat16
F8 = mybir.dt.float8e4
AF = mybir.ActivationFunctionType
DRS = mybir.MatmulPerfMode.DoubleRowSwInterleave
MUL = mybir.AluOpType.mult
ADD = mybir.AluOpType.add

NP_BF16 = ml_dtypes.bfloat16
NP_F8 = ml_dtypes.float8_e4m3

ENCV = D + NB  # merged fp8 value stream width


def build_program(with_bias: bool, repeats: int = 1):
    nc = bacc.Bacc("TRN2", target_bir_lowering=False, debug=False)
    encv_d = nc.dram_tensor("encv", [ROWS, ENCV], F8, kind="ExternalInput")
    encT_d = nc.dram_tensor("encT", [NT, P, KC, P], BF16, kind="ExternalInput")
    encT8_d = nc.dram_tensor("encT8", [NT, P, KC // 2, 2 * P], F8, kind="ExternalInput")
    wb_d = nc.dram_tensor("wb", [P, KC, NB], BF16, kind="ExternalInput")
    w8_d = nc.dram_tensor("w8", [P, KC, NF], F8, kind="ExternalInput")
    bb_d = b8_d = None
    if with_bias:
        bb_d = nc.dram_tensor("bb", [1, NB], BF16, kind="ExternalInput")
        b8_d = nc.dram_tensor("b8", [1, NF], BF16, kind="ExternalInput")
    out_d = nc.dram_tensor("out", [BPC, D], F32, kind="ExternalOutput")

    NTOT = repeats * NT

    with tile.TileContext(nc) as tc:
        with (
            tc.tile_pool(name="consts", bufs=1) as consts,
            tc.tile_pool(name="wpool", bufs=1) as wpool,
            tc.tile_pool(name="encvp", bufs=8) as encvp,
            tc.tile_pool(name="encTp", bufs=7) as encTp,
            tc.tile_pool(name="encT8p", bufs=7) as encT8p,
            tc.tile_pool(name="valhip", bufs=3) as valhip,
            tc.tile_pool(name="expp", bufs=3) as expp,
            tc.tile_pool(name="wtp", bufs=3) as wtp,
            tc.tile_pool(name="wt8p", bufs=3) as wt8p,
            tc.tile_pool(name="smalls", bufs=12) as smalls,
            tc.tile_pool(name="outp", bufs=2) as outp,
            tc.tile_pool(name="pr_ps", bufs=2, space=bass.MemorySpace.PSUM) as pr_ps,
            tc.tile_pool(name="ctx_ps", bufs=2, space=bass.MemorySpace.PSUM) as ctx_ps,
        ):
            # three DMA streams per tile, issue spread over SP, ScalarE and
            # Pool queues to avoid serializing on one descriptor queue
            loaded = {}

            def load(t):
                encv_t = encvp.tile([P, ENCV], F8, name="encv_t")
                nc.sync.dma_start(
                    encv_t, encv_d.ap()[(t % NT) * P : (t % NT + 1) * P, :]
                )
                encT = encTp.tile([P, KC, P], BF16, name="encT")
                nc.gpsimd.dma_start(encT, encT_d.ap()[t % NT])
                encT8 = encT8p.tile([P, KC // 2, 2 * P], F8, name="encT8")
                nc.scalar.dma_start(encT8, encT8_d.ap()[t % NT])
                loaded[t] = (encv_t, encT, encT8)

            state = {"ctx_half": None, "ones_col": None, "ones8": None}
            NPAIR = TPB // 2

            def emit_ctx(prev):
                # ctx for a PAIR of row tiles: the high-|dec| NB columns via
                # two bf16 matmuls (one per tile, out partition 0); the low
                # 768 via fp8 DoubleRowSwInterleave contracting both tiles'
                # 256 rows per pass.  The fp8 ones stationary is padded to
                # M=128 (full 256 active cols) -> 128 identical output
                # rows; row 0 is read.
                wt_hi2, wt82, pib, bidx = prev
                if pib == 0:
                    state["ctx_half"] = [
                        ctx_ps.tile([P, 512], F32, name=f"ctxh{h}") for h in range(2)
                    ]
                ctxh0, ctxh1 = state["ctx_half"]
                last = pib == NPAIR - 1
                for i in range(2):
                    nc.tensor.matmul(
                        ctxh0[0:1, 0:NB],
                        state["ones_col"],
                        wt_hi2[:, i, :],
                        start=(pib == 0 and i == 0),
                        stop=False,
                        skip_group_check=True,
                    )
                nc.tensor.matmul(
                    ctxh0[:, NB:512],
                    state["ones8"],
                    wt82[:, :, 0 : 512 - NB],
                    start=False,
                    stop=last,
                    perf_mode=DRS,
                    skip_group_check=True,
                )
                nc.tensor.matmul(
                    ctxh1,
                    state["ones8"],
                    wt82[:, :, 512 - NB : NF],
                    start=(pib == 0),
                    stop=last,
                    perf_mode=DRS,
                    skip_group_check=True,
                )
                if last:
                    # the two PSUM->SBUF drains run on DVE and ACT in
                    # parallel (both are [1,512] single-lane copies)
                    ctx_sb = outp.tile([1, D], F32, name="ctx_sb")
                    nc.vector.tensor_copy(
                        ctx_sb[:, 0:512], state["ctx_half"][0][0:1, :]
                    )
                    nc.scalar.add(
                        ctx_sb[:, 512:1024], state["ctx_half"][1][0:1, :], 0.0
                    )
                    nc.sync.dma_start(out_d.ap()[bidx : bidx + 1, :], ctx_sb)

            # load(0) first so tile 0's streams lead their queues; the W
            # DMAs follow split across the SP and ScalarE queues (the fp8
            # half used by the first two k-pairs rides ScalarE right after
            # encT8(0)) so the first-tile matmuls start ~2us in
            load(0)
            wb_sb = wpool.tile([P, KC, NB], BF16)
            nc.sync.dma_start(wb_sb, wb_d.ap())
            w8_sb = wpool.tile([P, KC, NF], F8)
            nc.scalar.dma_start(w8_sb[:, 0 : KC // 2, :], w8_d.ap()[:, 0 : KC // 2, :])
            nc.sync.dma_start(
                w8_sb[:, KC // 2 : KC, :], w8_d.ap()[:, KC // 2 : KC, :]
            )
            load(1)
            load(2)
            load(3)
            load(4)

            # constant ones stationaries for the ctx (sequence-sum) matmuls;
            # the softmax reciprocal is folded into wt on DVE instead
            ones_f = consts.tile([P, 1], F32)
            nc.any.memset(ones_f, 1.0)
            ones_col = consts.tile([P, 1], BF16)
            nc.vector.tensor_copy(ones_col, ones_f)
            state["ones_col"] = ones_col
            ones8_f = consts.tile([P, 2 * P], F32)
            nc.any.memset(ones8_f, 1.0)
            ones8 = consts.tile([P, 2 * P], F8)
            nc.vector.tensor_copy(ones8, ones8_f)
            state["ones8"] = ones8

            ones_row = None
            bb_sb = b8_sb = None
            if with_bias:
                ones_f32 = consts.tile([1, P], F32)
                nc.any.memset(ones_f32, 1.0)
                ones_row = consts.tile([1, P], BF16)
                nc.vector.tensor_copy(ones_row, ones_f32)
                bb_sb = consts.tile([1, NB], BF16)
                nc.sync.dma_start(bb_sb, bb_d.ap())
                b8_sb = consts.tile([1, NF], BF16)
                nc.sync.dma_start(b8_sb, b8_d.ap())

            pending = deque()
            for t in range(NTOT):
                bidx, tib = divmod(t % NT, TPB)
                if t + 5 < NTOT:
                    load(t + 5)
                encv_t, encT, encT8 = loaded.pop(t)

                # score layout (permuted channels): cols 0:NB bf16 part,
                # NB:1024 fp8 part.  One 2-bank PSUM tile; bank0 holds
                # [bf16 256 | fp8 256], bank1 holds fp8 512.
                pr = pr_ps.tile([P, 1024], F32, name="pr")

                # bf16 and fp8 DoubleRow parts interleaved per k-pair: each
                # DRS LDWEIGHTS follows two bf16 moving passes, giving the
                # weight-load pipeline something to hide under.  Flags are
                # order-independent (disjoint PSUM column regions with
                # first-touch via has_written).
                for c in range(KC // 2):
                    if c == 2 and len(pending) >= 2:
                        emit_ctx(pending.popleft())
                    # near the end of the program drain the ctx backlog
                    # eagerly so the post-loop tail stays short
                    if c == 0 and t + 4 >= NTOT and pending:
                        emit_ctx(pending.popleft())
                    ks = slice(2 * c, 2 * c + 2)
                    last = c == KC // 2 - 1
                    for k in (2 * c, 2 * c + 1):
                        nc.tensor.matmul(
                            pr[:, 0:NB],
                            encT[:, k, :],
                            wb_sb[:, k, :],
                            start=(k == 0),
                            stop=False,
                            skip_group_check=True,
                        )
                    nc.tensor.matmul(
                        pr[:, NB:512],
                        encT8[:, c, :],
                        w8_sb[:, ks, 0 : 512 - NB],
                        start=False,
                        stop=(last and not with_bias),
                        perf_mode=DRS,
                        skip_group_check=True,
                    )
                    nc.tensor.matmul(
                        pr[:, 512:1024],
                        encT8[:, c, :],
                        w8_sb[:, ks, 512 - NB : NF],
                        start=(c == 0),
                        stop=(last and not with_bias),
                        perf_mode=DRS,
                        skip_group_check=True,
                    )
                if with_bias:
                    nc.tensor.matmul(
                        pr[:, 0:NB],
                        ones_row,
                        bb_sb,
                        start=False,
                        stop=False,
                        skip_group_check=True,
                    )
                    nc.tensor.matmul(
                        pr[:, NB:512],
                        ones_row,
                        b8_sb[:, 0 : 512 - NB],
                        start=False,
                        stop=True,
                        skip_group_check=True,
                    )
                    nc.tensor.matmul(
                        pr[:, 512:1024],
                        ones_row,
                        b8_sb[:, 512 - NB : NF],
                        start=False,
                        stop=True,
                        skip_group_check=True,
                    )

                # single exp over both banks with fused row-sum; the whole W
                # carries the W8SCALE pre-scale (exact power of 2), undone here
                ssum = smalls.tile([P, 1], F32)
                exp_t = expp.tile([P, D], BF16)
                nc.scalar.activation(
                    exp_t, pr, AF.Exp, scale=1.0 / W8SCALE, accum_out=ssum
                )

                recip_f = smalls.tile([P, 1], F32)
                nc.vector.reciprocal(recip_f, ssum)
                recip64 = smalls.tile([P, 1], F32)
                nc.vector.tensor_scalar_mul(recip64, recip_f, WT8SCALE)

                # reconstruct the bf16 high-|dec| values: enc8 + er8
                vals_hi = valhip.tile([P, NB], BF16, name="vals_hi")
                nc.vector.scalar_tensor_tensor(
                    vals_hi, encv_t[:, 0:NB], 1.0, encv_t[:, D:ENCV],
                    op0=MUL, op1=ADD,
                )

                # wt = (exp * 1/rowsum) * enc fused on DVE, written into
                # tile-PAIR buffers: bf16 for the high-|dec| block, fp8
                # (x64 pre-scaled) for the rest
                if t % 2 == 0:
                    state["wt_hi2"] = wtp.tile([P, 2, NB], BF16, name="wt_hi2")
                    state["wt82"] = wt8p.tile([P, 2, NF], F8, name="wt82")
                half = t % 2
                nc.vector.scalar_tensor_tensor(
                    state["wt_hi2"][:, half, :], exp_t[:, 0:NB], recip_f,
                    vals_hi,
                    op0=MUL, op1=MUL,
                )
                nc.vector.scalar_tensor_tensor(
                    state["wt82"][:, half, :], exp_t[:, NB:D], recip64,
                    encv_t[:, NB:D],
                    op0=MUL, op1=MUL,
                )

                if t % 2 == 1:
                    pending.append(
                        (state["wt_hi2"], state["wt82"], (tib // 2), bidx)
                    )
            while pending:
                emit_ctx(pending.popleft())

    nc.compile()
    return nc


def _perm(dec):
    return np.argsort(-np.abs(dec), kind="stable")


def make_in_maps(hidden_dec, hidden_enc, W, b):
    enc = np.asarray(hidden_enc, dtype=np.float32).reshape(B, S, D)
    W = np.asarray(W, dtype=np.float32)
    dec = np.asarray(hidden_dec, dtype=np.float32).reshape(D)
    b = np.asarray(b, dtype=np.float32).reshape(D)
    with_bias = bool(np.any(b != 0.0))

    perm = _perm(dec)
    Weff = W * dec[None, :]
    Wp = Weff[np.ix_(perm, perm)]
    wb = np.ascontiguousarray(
        (Wp[:, :NB] * W8SCALE).reshape(KC, P, NB).transpose(1, 0, 2)
    ).astype(NP_BF16)
    w8 = np.ascontiguousarray(
        (Wp[:, NB:] * W8SCALE).reshape(KC, P, NF).transpose(1, 0, 2)
    ).astype(NP_F8)
    encp = enc[:, :, perm].astype(NP_BF16)

    b_eff = (b * dec)[perm] * W8SCALE
    bb = b_eff[:NB].reshape(1, NB).astype(NP_BF16)
    b8 = b_eff[NB:].reshape(1, NF).astype(NP_BF16)

    in_maps = []
    for c in range(NCORES):
        ev = encp[c * BPC : (c + 1) * BPC].reshape(ROWS, D)
        ev8 = ev.astype(NP_F8)
        er8_hi = (
            ev[:, :NB].astype(np.float32) - ev8[:, :NB].astype(np.float32)
        ).astype(NP_F8)
        encv = np.concatenate([ev8, er8_hi], axis=1)
        # host-side tiled transpose into the exact SBUF stationary layout:
        # encT[t, p, kc, r] = enc[t*128 + r, kc*128 + p]
        encT = np.ascontiguousarray(
            ev.reshape(NT, P, KC, P).transpose(0, 3, 2, 1)
        )
        # fp8 copy, software-interleaved for DoubleRowSwInterleave: per
        # partition each k-pair's stationary stream is
        # [A_col127, B_col127, A_col126, ..., B_col0] (A/B = the two
        # k-chunks, columns reversed)
        e8rev = encT.astype(NP_F8)[:, :, :, ::-1]
        enc8i = np.ascontiguousarray(
            e8rev.reshape(NT, P, KC // 2, 2, P).transpose(0, 1, 2, 4, 3)
        ).reshape(NT, P, KC // 2, 2 * P)
        m = {
            "encv": np.ascontiguousarray(encv),
            "encT": encT,
            "encT8": enc8i,
            "wb": wb,
            "w8": w8,
        }
        if with_bias:
            m["bb"] = bb
            m["b8"] = b8
        in_maps.append(m)
    return in_maps, with_bias


def kernel(hidden_dec, hidden_enc, W, b):
    in_maps, with_bias = make_in_maps(hidden_dec, hidden_enc, W, b)
    nc = build_program(with_bias)
    res = bass_utils.run_bass_kernel_spmd(nc, in_maps, core_ids=list(range(NCORES)))
    outp = np.concatenate([res.results[c]["out"] for c in range(NCORES)], axis=0)
    outp[:, NB:] /= WT8SCALE  # undo the fp8 wt pre-scale on the low block
    perm = _perm(np.asarray(hidden_dec, dtype=np.float32).reshape(D))
    out = np.empty_like(outp)
    out[:, perm] = outp
    return out.astype(np.float32)


# revision 17
# speedup vs baseline: 1.0283x; 1.0172x over previous
"""Trainium2 Bass kernel for nn_Attention_75814762709205.

Computation (per batch row b, seq s):
    proj  = hidden_enc @ W + b          [B,S,D]
    score = hidden_dec.T * proj         (per-channel scale)
    attn  = softmax(score, axis=-1)     (over D)
    out   = sum_s attn * hidden_enc     [B,D]

Sharding: data-parallel over batch, 4 batches per core on 8 cores.

Precision strategy (validated numerically against the exact reference
data): the softmax error from an fp8 matmul is amplified by exp() in
proportion to |dec_e|, so the channels are permuted by |dec| on the
host.  The top NB=256 channels go through a bf16 matmul; the remaining
NF=768 channels go through an fp8e4 DoubleRow matmul.  dec is folded
into W on the host (score = enc @ (W*dec)).

V4 data-stream layout: the attention VALUES arrive as one fp8 byte
stream encv = [enc8 | er8_hi] (enc8 = fp8(enc); er8 = fp8(enc - enc8)
for the top NB cols).  The top-NB bf16 values are reconstructed on DVE
as enc8 + er8 (0.2% error, better than bf16); the low-channel values
are used in fp8 directly (they feed the x64-scaled fp8 ctx matmul).
This cuts the value stream from 16MB bf16 to 10MB fp8 and keeps one
DMA per stream-tile.  The bf16 transposed enc (matmul stationary) and
the fp8 SW-interleaved copy are host-prepared as before.

Softmax denominator is folded into the final sequence reduction:
context = sum_s r_s * (exp_s * enc_s) with r = 1/sum(exp).  The ctx
matmuls run two tiles behind so the PE never waits on the ACT/DVE
softmax tail.  PSUM: 2 pr bufs (2 banks each) + 2 ctx bufs (2 banks
each) so consecutive batches' ctx accumulators don't serialize.
"""

import sys

sys.path.insert(0, "/opt/trn_rl_repo")

from collections import deque

import numpy as np
import ml_dtypes

import concourse.bass as bass
import concourse.mybir as mybir
import concourse.tile as tile
from concourse import bacc, bass_utils

B, S, D = 32, 2048, 1024
NCORES = 8
BPC = B // NCORES  # batches per core
ROWS = BPC * S  # rows per core
P = 128
NT = ROWS // P  # row tiles per core
TPB = S // P  # row tiles per batch
KC = D // P  # contraction chunks
NB = 96  # bf16 (high |dec|) channels
NF = D - NB  # fp8 channels
W8SCALE = 8.0  # fp8 W pre-scale (undone in the exp activation)
WT8SCALE = 64.0  # fp8 wt pre-scale (keeps softmax weights in normal range;
                 # undone on the host after the output gather)

F32 = mybir.dt.float32
BF16 = mybir.dt.bfloat16
F8 = mybir.dt.float8e4
AF = mybir.ActivationFunctionType
DRS = mybir.MatmulPerfMode.DoubleRowSwInterleave
MUL = mybir.AluOpType.mult
ADD = mybir.AluOpType.add

NP_BF16 = ml_dtypes.bfloat16
NP_F8 = ml_dtypes.float8_e4m3

ENCV = D + NB  # merged fp8 value stream width


def build_program(with_bias: bool, repeats: int = 1):
    nc = bacc.Bacc("TRN2", target_bir_lowering=False, debug=False)
    encv_d = nc.dram_tensor("encv", [ROWS, ENCV], F8, kind="ExternalInput")
    encT_d = nc.dram_tensor("encT", [NT, P, KC, P], BF16, kind="ExternalInput")
    encT8_d = nc.dram_tensor("encT8", [NT, P, KC // 2, 2 * P], F8, kind="ExternalInput")
    wb_d = nc.dram_tensor("wb", [P, KC, NB], BF16, kind="ExternalInput")
    w8_d = nc.dram_tensor("w8", [P, KC, NF], F8, kind="ExternalInput")
    bb_d = b8_d = None
    if with_bias:
        bb_d = nc.dram_tensor("bb", [1, NB], BF16, kind="ExternalInput")
        b8_d = nc.dram_tensor("b8", [1, NF], BF16, kind="ExternalInput")
    out_d = nc.dram_tensor("out", [BPC, D], F32, kind="ExternalOutput")

    NTOT = repeats * NT

    with tile.TileContext(nc) as tc:
        with (
            tc.tile_pool(name="consts", bufs=1) as consts,
            tc.tile_pool(name="wpool", bufs=1) as wpool,
            tc.tile_pool(name="encvp", bufs=6) as encvp,
            tc.tile_pool(name="encTp", bufs=5) as encTp,
            tc.tile_pool(name="encT8p", bufs=5) as encT8p,
            tc.tile_pool(name="valhip", bufs=3) as valhip,
            tc.tile_pool(name="expp", bufs=3) as expp,
            tc.tile_pool(name="wtp", bufs=3) as wtp,
            tc.tile_pool(name="wt8p", bufs=3) as wt8p,
            tc.tile_pool(name="smalls", bufs=12) as smalls,
            tc.tile_pool(name="outp", bufs=2) as outp,
            tc.tile_pool(name="pr_ps", bufs=2, space=bass.MemorySpace.PSUM) as pr_ps,
            tc.tile_pool(name="ctx_ps", bufs=2, space=bass.MemorySpace.PSUM) as ctx_ps,
        ):
            # ---- W setup first so the weight DMAs overlap the enc
            # bootstrap instead of queueing behind it ----
            wb_sb = wpool.tile([P, KC, NB], BF16)
            nc.sync.dma_start(wb_sb, wb_d.ap())
            w8_sb = wpool.tile([P, KC, NF], F8)
            nc.scalar.dma_start(w8_sb, w8_d.ap())

            # three DMA streams per tile, issue spread over SP, ScalarE and
            # Pool queues to avoid serializing on one descriptor queue
            loaded = {}

            def load(t):
                encv_t = encvp.tile([P, ENCV], F8, name="encv_t")
                nc.sync.dma_start(
                    encv_t, encv_d.ap()[(t % NT) * P : (t % NT + 1) * P, :]
                )
                encT = encTp.tile([P, KC, P], BF16, name="encT")
                nc.gpsimd.dma_start(encT, encT_d.ap()[t % NT])
                encT8 = encT8p.tile([P, KC // 2, 2 * P], F8, name="encT8")
                nc.scalar.dma_start(encT8, encT8_d.ap()[t % NT])
                loaded[t] = (encv_t, encT, encT8)

            state = {"ctx_half": None, "ones_col": None, "ones8": None}
            NPAIR = TPB // 2

            def emit_ctx(prev):
                # ctx for a PAIR of row tiles: the high-|dec| NB columns via
                # two bf16 matmuls (one per tile, out partition 0); the low
                # 768 via fp8 DoubleRowSwInterleave contracting both tiles'
                # 256 rows per pass.  The fp8 ones stationary is padded to
                # M=128 (full 256 active cols) -> 128 identical output
                # rows; row 0 is read.
                wt_hi2, wt82, pib, bidx = prev
                if pib == 0:
                    state["ctx_half"] = [
                        ctx_ps.tile([P, 512], F32, name=f"ctxh{h}") for h in range(2)
                    ]
                ctxh0, ctxh1 = state["ctx_half"]
                last = pib == NPAIR - 1
                for i in range(2):
                    nc.tensor.matmul(
                        ctxh0[0:1, 0:NB],
                        state["ones_col"],
                        wt_hi2[:, i, :],
                        start=(pib == 0 and i == 0),
                        stop=False,
                        skip_group_check=True,
                    )
                nc.tensor.matmul(
                    ctxh0[:, NB:512],
                    state["ones8"],
                    wt82[:, :, 0 : 512 - NB],
                    start=False,
                    stop=last,
                    perf_mode=DRS,
                    skip_group_check=True,
                )
                nc.tensor.matmul(
                    ctxh1,
                    state["ones8"],
                    wt82[:, :, 512 - NB : NF],
                    start=(pib == 0),
                    stop=last,
                    perf_mode=DRS,
                    skip_group_check=True,
                )
                if last:
                    ctx_sb = outp.tile([1, D], F32, name="ctx_sb")
                    for h2 in range(2):
                        nc.vector.tensor_copy(
                            ctx_sb[:, h2 * 512 : (h2 + 1) * 512],
                            state["ctx_half"][h2][0:1, :],
                        )
                    nc.sync.dma_start(out_d.ap()[bidx : bidx + 1, :], ctx_sb)

            load(0)
            load(1)
            load(2)

            # constant ones stationaries for the ctx (sequence-sum) matmuls;
            # the softmax reciprocal is folded into wt on DVE instead
            ones_f = consts.tile([P, 1], F32)
            nc.any.memset(ones_f, 1.0)
            ones_col = consts.tile([P, 1], BF16)
            nc.vector.tensor_copy(ones_col, ones_f)
            state["ones_col"] = ones_col
            ones8_f = consts.tile([P, 2 * P], F32)
            nc.any.memset(ones8_f, 1.0)
            ones8 = consts.tile([P, 2 * P], F8)
            nc.vector.tensor_copy(ones8, ones8_f)
            state["ones8"] = ones8

            ones_row = None
            bb_sb = b8_sb = None
            if with_bias:
                ones_f32 = consts.tile([1, P], F32)
                nc.any.memset(ones_f32, 1.0)
                ones_row = consts.tile([1, P], BF16)
                nc.vector.tensor_copy(ones_row, ones_f32)
                bb_sb = consts.tile([1, NB], BF16)
                nc.sync.dma_start(bb_sb, bb_d.ap())
                b8_sb = consts.tile([1, NF], BF16)
                nc.sync.dma_start(b8_sb, b8_d.ap())

            pending = deque()
            for t in range(NTOT):
                bidx, tib = divmod(t % NT, TPB)
                if t + 3 < NTOT:
                    load(t + 3)
                encv_t, encT, encT8 = loaded.pop(t)

                # score layout (permuted channels): cols 0:NB bf16 part,
                # NB:1024 fp8 part.  One 2-bank PSUM tile; bank0 holds
                # [bf16 256 | fp8 256], bank1 holds fp8 512.
                pr = pr_ps.tile([P, 1024], F32, name="pr")

                # bf16 part: shares bank0 with the first fp8 region
                # (disjoint columns; the fp8 matmuls use start=False and
                # overwrite-on-first-touch via has_written)
                for k in range(KC):
                    nc.tensor.matmul(
                        pr[:, 0:NB],
                        encT[:, k, :],
                        wb_sb[:, k, :],
                        start=(k == 0),
                        stop=False,
                        skip_group_check=True,
                    )
                # fp8 DoubleRow part (software-interleaved stationary: the
                # host lays each k-pair out contiguously, so LDWEIGHTS is a
                # linear read instead of the HW interleave pattern)
                for c in range(KC // 2):
                    if c == 2 and len(pending) >= 2:
                        emit_ctx(pending.popleft())
                    ks = slice(2 * c, 2 * c + 2)
                    last = c == KC // 2 - 1
                    nc.tensor.matmul(
                        pr[:, NB:512],
                        encT8[:, c, :],
                        w8_sb[:, ks, 0 : 512 - NB],
                        start=False,
                        stop=(last and not with_bias),
                        perf_mode=DRS,
                        skip_group_check=True,
                    )
                    nc.tensor.matmul(
                        pr[:, 512:1024],
                        encT8[:, c, :],
                        w8_sb[:, ks, 512 - NB : NF],
                        start=(c == 0),
                        stop=(last and not with_bias),
                        perf_mode=DRS,
                        skip_group_check=True,
                    )
                if with_bias:
                    nc.tensor.matmul(
                        pr[:, 0:NB],
                        ones_row,
                        bb_sb,
                        start=False,
                        stop=False,
                        skip_group_check=True,
                    )
                    nc.tensor.matmul(
                        pr[:, NB:512],
                        ones_row,
                        b8_sb[:, 0 : 512 - NB],
                        start=False,
                        stop=True,
                        skip_group_check=True,
                    )
                    nc.tensor.matmul(
                        pr[:, 512:1024],
                        ones_row,
                        b8_sb[:, 512 - NB : NF],
                        start=False,
                        stop=True,
                        skip_group_check=True,
                    )

                # single exp over both banks with fused row-sum; the whole W
                # carries the W8SCALE pre-scale (exact power of 2), undone here
                ssum = smalls.tile([P, 1], F32)
                exp_t = expp.tile([P, D], BF16)
                nc.scalar.activation(
                    exp_t, pr, AF.Exp, scale=1.0 / W8SCALE, accum_out=ssum
                )

                recip_f = smalls.tile([P, 1], F32)
                nc.vector.reciprocal(recip_f, ssum)
                recip64 = smalls.tile([P, 1], F32)
                nc.vector.tensor_scalar_mul(recip64, recip_f, WT8SCALE)

                # reconstruct the bf16 high-|dec| values: enc8 + er8
                vals_hi = valhip.tile([P, NB], BF16, name="vals_hi")
                nc.vector.scalar_tensor_tensor(
                    vals_hi, encv_t[:, 0:NB], 1.0, encv_t[:, D:ENCV],
                    op0=MUL, op1=ADD,
                )

                # wt = (exp * 1/rowsum) * enc fused on DVE, written into
                # tile-PAIR buffers: bf16 for the high-|dec| block, fp8
                # (x64 pre-scaled) for the rest
                if t % 2 == 0:
                    state["wt_hi2"] = wtp.tile([P, 2, NB], BF16, name="wt_hi2")
                    state["wt82"] = wt8p.tile([P, 2, NF], F8, name="wt82")
                half = t % 2
                nc.vector.scalar_tensor_tensor(
                    state["wt_hi2"][:, half, :], exp_t[:, 0:NB], recip_f,
                    vals_hi,
                    op0=MUL, op1=MUL,
                )
                nc.vector.scalar_tensor_tensor(
                    state["wt82"][:, half, :], exp_t[:, NB:D], recip64,
                    encv_t[:, NB:D],
                    op0=MUL, op1=MUL,
                )

                if t % 2 == 1:
                    pending.append(
                        (state["wt_hi2"], state["wt82"], (tib // 2), bidx)
                    )
            while pending:
                emit_ctx(pending.popleft())

    nc.compile()
    return nc


def _perm(dec):
    return np.argsort(-np.abs(dec), kind="stable")


def make_in_maps(hidden_dec, hidden_enc, W, b):
    enc = np.asarray(hidden_enc, dtype=np.float32).reshape(B, S, D)
    W = np.asarray(W, dtype=np.float32)
    dec = np.asarray(hidden_dec, dtype=np.float32).reshape(D)
    b = np.asarray(b, dtype=np.float32).reshape(D)
    with_bias = bool(np.any(b != 0.0))

    perm = _perm(dec)
    Weff = W * dec[None, :]
    Wp = Weff[np.ix_(perm, perm)]
    wb = np.ascontiguousarray(
        (Wp[:, :NB] * W8SCALE).reshape(KC, P, NB).transpose(1, 0, 2)
    ).astype(NP_BF16)
    w8 = np.ascontiguousarray(
        (Wp[:, NB:] * W8SCALE).reshape(KC, P, NF).transpose(1, 0, 2)
    ).astype(NP_F8)
    encp = enc[:, :, perm].astype(NP_BF16)

    b_eff = (b * dec)[perm] * W8SCALE
    bb = b_eff[:NB].reshape(1, NB).astype(NP_BF16)
    b8 = b_eff[NB:].reshape(1, NF).astype(NP_BF16)

    in_maps = []
    for c in range(NCORES):
        ev = encp[c * BPC : (c + 1) * BPC].reshape(ROWS, D)
        ev8 = ev.astype(NP_F8)
        er8_hi = (
            ev[:, :NB].astype(np.float32) - ev8[:, :NB].astype(np.float32)
        ).astype(NP_F8)
        encv = np.concatenate([ev8, er8_hi], axis=1)
        # host-side tiled transpose into the exact SBUF stationary layout:
        # encT[t, p, kc, r] = enc[t*128 + r, kc*128 + p]
        encT = np.ascontiguousarray(
            ev.reshape(NT, P, KC, P).transpose(0, 3, 2, 1)
        )
        # fp8 copy, software-interleaved for DoubleRowSwInterleave: per
        # partition each k-pair's stationary stream is
        # [A_col127, B_col127, A_col126, ..., B_col0] (A/B = the two
        # k-chunks, columns reversed)
        e8rev = encT.astype(NP_F8)[:, :, :, ::-1]
        enc8i = np.ascontiguousarray(
            e8rev.reshape(NT, P, KC // 2, 2, P).transpose(0, 1, 2, 4, 3)
        ).reshape(NT, P, KC // 2, 2 * P)
        m = {
            "encv": np.ascontiguousarray(encv),
            "encT": encT,
            "encT8": enc8i,
            "wb": wb,
            "w8": w8,
        }
        if with_bias:
            m["bb"] = bb
            m["b8"] = b8
        in_maps.append(m)
    return in_maps, with_bias


def kernel(hidden_dec, hidden_enc, W, b):
    in_maps, with_bias = make_in_maps(hidden_dec, hidden_enc, W, b)
    nc = build_program(with_bias)
    res = bass_utils.run_bass_kernel_spmd(nc, in_maps, core_ids=list(range(NCORES)))
    outp = np.concatenate([res.results[c]["out"] for c in range(NCORES)], axis=0)
    outp[:, NB:] /= WT8SCALE  # undo the fp8 wt pre-scale on the low block
    perm = _perm(np.asarray(hidden_dec, dtype=np.float32).reshape(D))
    out = np.empty_like(outp)
    out[:, perm] = outp
    return out.astype(np.float32)


# revision 18
# speedup vs baseline: 1.0292x; 1.0008x over previous
"""Trainium2 Bass kernel for nn_Attention_75814762709205.

Computation (per batch row b, seq s):
    proj  = hidden_enc @ W + b          [B,S,D]
    score = hidden_dec.T * proj         (per-channel scale)
    attn  = softmax(score, axis=-1)     (over D)
    out   = sum_s attn * hidden_enc     [B,D]

Sharding: data-parallel over batch, 4 batches per core on 8 cores.

Precision strategy (validated numerically against the exact reference
data): the softmax error from an fp8 matmul is amplified by exp() in
proportion to |dec_e|, so the channels are permuted by |dec| on the
host.  The top NB=256 channels go through a bf16 matmul; the remaining
NF=768 channels go through an fp8e4 DoubleRow matmul.  dec is folded
into W on the host (score = enc @ (W*dec)).

V4 data-stream layout: the attention VALUES arrive as one fp8 byte
stream encv = [enc8 | er8_hi] (enc8 = fp8(enc); er8 = fp8(enc - enc8)
for the top NB cols).  The top-NB bf16 values are reconstructed on DVE
as enc8 + er8 (0.2% error, better than bf16); the low-channel values
are used in fp8 directly (they feed the x64-scaled fp8 ctx matmul).
This cuts the value stream from 16MB bf16 to 10MB fp8 and keeps one
DMA per stream-tile.  The bf16 transposed enc (matmul stationary) and
the fp8 SW-interleaved copy are host-prepared as before.

Softmax denominator is folded into the final sequence reduction:
context = sum_s r_s * (exp_s * enc_s) with r = 1/sum(exp).  The ctx
matmuls run two tiles behind so the PE never waits on the ACT/DVE
softmax tail.  PSUM: 2 pr bufs (2 banks each) + 2 ctx bufs (2 banks
each) so consecutive batches' ctx accumulators don't serialize.
"""

import sys

sys.path.insert(0, "/opt/trn_rl_repo")

from collections import deque

import numpy as np
import ml_dtypes

import concourse.bass as bass
import concourse.mybir as mybir
import concourse.tile as tile
from concourse import bacc, bass_utils

B, S, D = 32, 2048, 1024
NCORES = 8
BPC = B // NCORES  # batches per core
ROWS = BPC * S  # rows per core
P = 128
NT = ROWS // P  # row tiles per core
TPB = S // P  # row tiles per batch
KC = D // P  # contraction chunks
NB = 96  # bf16 (high |dec|) channels
NF = D - NB  # fp8 channels
W8SCALE = 8.0  # fp8 W pre-scale (undone in the exp activation)
WT8SCALE = 64.0  # fp8 wt pre-scale (keeps softmax weights in normal range;
                 # undone on the host after the output gather)

F32 = mybir.dt.float32
BF16 = mybir.dt.bfloat16
F8 = mybir.dt.float8e4
AF = mybir.ActivationFunctionType
DRS = mybir.MatmulPerfMode.DoubleRowSwInterleave
MUL = mybir.AluOpType.mult
ADD = mybir.AluOpType.add

NP_BF16 = ml_dtypes.bfloat16
NP_F8 = ml_dtypes.float8_e4m3

ENCV = D + NB  # merged fp8 value stream width


def build_program(with_bias: bool, repeats: int = 1):
    nc = bacc.Bacc("TRN2", target_bir_lowering=False, debug=False)
    encv_d = nc.dram_tensor("encv", [ROWS, ENCV], F8, kind="ExternalInput")
    encT_d = nc.dram_tensor("encT", [NT, P, KC, P], BF16, kind="ExternalInput")
    encT8_d = nc.dram_tensor("encT8", [NT, P, KC // 2, 2 * P], F8, kind="ExternalInput")
    wb_d = nc.dram_tensor("wb", [P, KC, NB], BF16, kind="ExternalInput")
    w8_d = nc.dram_tensor("w8", [P, KC, NF], F8, kind="ExternalInput")
    bb_d = b8_d = None
    if with_bias:
        bb_d = nc.dram_tensor("bb", [1, NB], BF16, kind="ExternalInput")
        b8_d = nc.dram_tensor("b8", [1, NF], BF16, kind="ExternalInput")
    out_d = nc.dram_tensor("out", [BPC, D], F32, kind="ExternalOutput")

    NTOT = repeats * NT

    with tile.TileContext(nc) as tc:
        with (
            tc.tile_pool(name="consts", bufs=1) as consts,
            tc.tile_pool(name="wpool", bufs=1) as wpool,
            tc.tile_pool(name="encvp", bufs=6) as encvp,
            tc.tile_pool(name="encTp", bufs=5) as encTp,
            tc.tile_pool(name="encT8p", bufs=5) as encT8p,
            tc.tile_pool(name="valhip", bufs=3) as valhip,
            tc.tile_pool(name="expp", bufs=3) as expp,
            tc.tile_pool(name="wtp", bufs=3) as wtp,
            tc.tile_pool(name="wt8p", bufs=3) as wt8p,
            tc.tile_pool(name="smalls", bufs=12) as smalls,
            tc.tile_pool(name="outp", bufs=2) as outp,
            tc.tile_pool(name="pr_ps", bufs=2, space=bass.MemorySpace.PSUM) as pr_ps,
            tc.tile_pool(name="ctx_ps", bufs=2, space=bass.MemorySpace.PSUM) as ctx_ps,
        ):
            # ---- W setup first so the weight DMAs overlap the enc
            # bootstrap instead of queueing behind it ----
            wb_sb = wpool.tile([P, KC, NB], BF16)
            nc.sync.dma_start(wb_sb, wb_d.ap())
            w8_sb = wpool.tile([P, KC, NF], F8)
            nc.scalar.dma_start(w8_sb, w8_d.ap())

            # three DMA streams per tile, issue spread over SP, ScalarE and
            # Pool queues to avoid serializing on one descriptor queue
            loaded = {}

            def load(t):
                encv_t = encvp.tile([P, ENCV], F8, name="encv_t")
                nc.sync.dma_start(
                    encv_t, encv_d.ap()[(t % NT) * P : (t % NT + 1) * P, :]
                )
                # the big bf16 stationary stream rides the ScalarE HWDGE; the
                # smaller fp8 copy takes the Pool SWDGE (half the descriptor
                # generation load on the software path)
                encT = encTp.tile([P, KC, P], BF16, name="encT")
                nc.scalar.dma_start(encT, encT_d.ap()[t % NT])
                encT8 = encT8p.tile([P, KC // 2, 2 * P], F8, name="encT8")
                nc.gpsimd.dma_start(encT8, encT8_d.ap()[t % NT])
                loaded[t] = (encv_t, encT, encT8)

            state = {"ctx_half": None, "ones_col": None, "ones8": None}
            NPAIR = TPB // 2

            def emit_ctx(prev):
                # ctx for a PAIR of row tiles: the high-|dec| NB columns via
                # two bf16 matmuls (one per tile, out partition 0); the low
                # 768 via fp8 DoubleRowSwInterleave contracting both tiles'
                # 256 rows per pass.  The fp8 ones stationary is padded to
                # M=128 (full 256 active cols) -> 128 identical output
                # rows; row 0 is read.
                wt_hi2, wt82, pib, bidx = prev
                if pib == 0:
                    state["ctx_half"] = [
                        ctx_ps.tile([P, 512], F32, name=f"ctxh{h}") for h in range(2)
                    ]
                ctxh0, ctxh1 = state["ctx_half"]
                last = pib == NPAIR - 1
                for i in range(2):
                    nc.tensor.matmul(
                        ctxh0[0:1, 0:NB],
                        state["ones_col"],
                        wt_hi2[:, i, :],
                        start=(pib == 0 and i == 0),
                        stop=False,
                        skip_group_check=True,
                    )
                nc.tensor.matmul(
                    ctxh0[:, NB:512],
                    state["ones8"],
                    wt82[:, :, 0 : 512 - NB],
                    start=False,
                    stop=last,
                    perf_mode=DRS,
                    skip_group_check=True,
                )
                nc.tensor.matmul(
                    ctxh1,
                    state["ones8"],
                    wt82[:, :, 512 - NB : NF],
                    start=(pib == 0),
                    stop=last,
                    perf_mode=DRS,
                    skip_group_check=True,
                )
                if last:
                    ctx_sb = outp.tile([1, D], F32, name="ctx_sb")
                    for h2 in range(2):
                        nc.vector.tensor_copy(
                            ctx_sb[:, h2 * 512 : (h2 + 1) * 512],
                            state["ctx_half"][h2][0:1, :],
                        )
                    nc.sync.dma_start(out_d.ap()[bidx : bidx + 1, :], ctx_sb)

            load(0)
            load(1)
            load(2)

            # constant ones stationaries for the ctx (sequence-sum) matmuls;
            # the softmax reciprocal is folded into wt on DVE instead
            ones_f = consts.tile([P, 1], F32)
            nc.any.memset(ones_f, 1.0)
            ones_col = consts.tile([P, 1], BF16)
            nc.vector.tensor_copy(ones_col, ones_f)
            state["ones_col"] = ones_col
            ones8_f = consts.tile([P, 2 * P], F32)
            nc.any.memset(ones8_f, 1.0)
            ones8 = consts.tile([P, 2 * P], F8)
            nc.vector.tensor_copy(ones8, ones8_f)
            state["ones8"] = ones8

            ones_row = None
            bb_sb = b8_sb = None
            if with_bias:
                ones_f32 = consts.tile([1, P], F32)
                nc.any.memset(ones_f32, 1.0)
                ones_row = consts.tile([1, P], BF16)
                nc.vector.tensor_copy(ones_row, ones_f32)
                bb_sb = consts.tile([1, NB], BF16)
                nc.sync.dma_start(bb_sb, bb_d.ap())
                b8_sb = consts.tile([1, NF], BF16)
                nc.sync.dma_start(b8_sb, b8_d.ap())

            pending = deque()
            for t in range(NTOT):
                bidx, tib = divmod(t % NT, TPB)
                if t + 3 < NTOT:
                    load(t + 3)
                encv_t, encT, encT8 = loaded.pop(t)

                # score layout (permuted channels): cols 0:NB bf16 part,
                # NB:1024 fp8 part.  One 2-bank PSUM tile; bank0 holds
                # [bf16 256 | fp8 256], bank1 holds fp8 512.
                pr = pr_ps.tile([P, 1024], F32, name="pr")

                # bf16 part: shares bank0 with the first fp8 region
                # (disjoint columns; the fp8 matmuls use start=False and
                # overwrite-on-first-touch via has_written)
                for k in range(KC):
                    nc.tensor.matmul(
                        pr[:, 0:NB],
                        encT[:, k, :],
                        wb_sb[:, k, :],
                        start=(k == 0),
                        stop=False,
                        skip_group_check=True,
                    )
                # fp8 DoubleRow part (software-interleaved stationary: the
                # host lays each k-pair out contiguously, so LDWEIGHTS is a
                # linear read instead of the HW interleave pattern)
                for c in range(KC // 2):
                    if c == 2 and len(pending) >= 2:
                        emit_ctx(pending.popleft())
                    ks = slice(2 * c, 2 * c + 2)
                    last = c == KC // 2 - 1
                    nc.tensor.matmul(
                        pr[:, NB:512],
                        encT8[:, c, :],
                        w8_sb[:, ks, 0 : 512 - NB],
                        start=False,
                        stop=(last and not with_bias),
                        perf_mode=DRS,
                        skip_group_check=True,
                    )
                    nc.tensor.matmul(
                        pr[:, 512:1024],
                        encT8[:, c, :],
                        w8_sb[:, ks, 512 - NB : NF],
                        start=(c == 0),
                        stop=(last and not with_bias),
                        perf_mode=DRS,
                        skip_group_check=True,
                    )
                if with_bias:
                    nc.tensor.matmul(
                        pr[:, 0:NB],
                        ones_row,
                        bb_sb,
                        start=False,
                        stop=False,
                        skip_group_check=True,
                    )
                    nc.tensor.matmul(
                        pr[:, NB:512],
                        ones_row,
                        b8_sb[:, 0 : 512 - NB],
                        start=False,
                        stop=True,
                        skip_group_check=True,
                    )
                    nc.tensor.matmul(
                        pr[:, 512:1024],
                        ones_row,
                        b8_sb[:, 512 - NB : NF],
                        start=False,
                        stop=True,
                        skip_group_check=True,
                    )

                # single exp over both banks with fused row-sum; the whole W
                # carries the W8SCALE pre-scale (exact power of 2), undone here
                ssum = smalls.tile([P, 1], F32)
                exp_t = expp.tile([P, D], BF16)
                nc.scalar.activation(
                    exp_t, pr, AF.Exp, scale=1.0 / W8SCALE, accum_out=ssum
                )

                recip_f = smalls.tile([P, 1], F32)
                nc.vector.reciprocal(recip_f, ssum)
                recip64 = smalls.tile([P, 1], F32)
                nc.vector.tensor_scalar_mul(recip64, recip_f, WT8SCALE)

                # reconstruct the bf16 high-|dec| values: enc8 + er8
                vals_hi = valhip.tile([P, NB], BF16, name="vals_hi")
                nc.vector.scalar_tensor_tensor(
                    vals_hi, encv_t[:, 0:NB], 1.0, encv_t[:, D:ENCV],
                    op0=MUL, op1=ADD,
                )

                # wt = (exp * 1/rowsum) * enc fused on DVE, written into
                # tile-PAIR buffers: bf16 for the high-|dec| block, fp8
                # (x64 pre-scaled) for the rest
                if t % 2 == 0:
                    state["wt_hi2"] = wtp.tile([P, 2, NB], BF16, name="wt_hi2")
                    state["wt82"] = wt8p.tile([P, 2, NF], F8, name="wt82")
                half = t % 2
                nc.vector.scalar_tensor_tensor(
                    state["wt_hi2"][:, half, :], exp_t[:, 0:NB], recip_f,
                    vals_hi,
                    op0=MUL, op1=MUL,
                )
                nc.vector.scalar_tensor_tensor(
                    state["wt82"][:, half, :], exp_t[:, NB:D], recip64,
                    encv_t[:, NB:D],
                    op0=MUL, op1=MUL,
                )

                if t % 2 == 1:
                    pending.append(
                        (state["wt_hi2"], state["wt82"], (tib // 2), bidx)
                    )
            while pending:
                emit_ctx(pending.popleft())

    nc.compile()
    return nc


def _perm(dec):
    return np.argsort(-np.abs(dec), kind="stable")


def make_in_maps(hidden_dec, hidden_enc, W, b):
    enc = np.asarray(hidden_enc, dtype=np.float32).reshape(B, S, D)
    W = np.asarray(W, dtype=np.float32)
    dec = np.asarray(hidden_dec, dtype=np.float32).reshape(D)
    b = np.asarray(b, dtype=np.float32).reshape(D)
    with_bias = bool(np.any(b != 0.0))

    perm = _perm(dec)
    Weff = W * dec[None, :]
    Wp = Weff[np.ix_(perm, perm)]
    wb = np.ascontiguousarray(
        (Wp[:, :NB] * W8SCALE).reshape(KC, P, NB).transpose(1, 0, 2)
    ).astype(NP_BF16)
    w8 = np.ascontiguousarray(
        (Wp[:, NB:] * W8SCALE).reshape(KC, P, NF).transpose(1, 0, 2)
    ).astype(NP_F8)
    encp = enc[:, :, perm].astype(NP_BF16)

    b_eff = (b * dec)[perm] * W8SCALE
    bb = b_eff[:NB].reshape(1, NB).astype(NP_BF16)
    b8 = b_eff[NB:].reshape(1, NF).astype(NP_BF16)

    in_maps = []
    for c in range(NCORES):
        ev = encp[c * BPC : (c + 1) * BPC].reshape(ROWS, D)
        ev8 = ev.astype(NP_F8)
        er8_hi = (
            ev[:, :NB].astype(np.float32) - ev8[:, :NB].astype(np.float32)
        ).astype(NP_F8)
        encv = np.concatenate([ev8, er8_hi], axis=1)
        # host-side tiled transpose into the exact SBUF stationary layout:
        # encT[t, p, kc, r] = enc[t*128 + r, kc*128 + p]
        encT = np.ascontiguousarray(
            ev.reshape(NT, P, KC, P).transpose(0, 3, 2, 1)
        )
        # fp8 copy, software-interleaved for DoubleRowSwInterleave: per
        # partition each k-pair's stationary stream is
        # [A_col127, B_col127, A_col126, ..., B_col0] (A/B = the two
        # k-chunks, columns reversed)
        e8rev = encT.astype(NP_F8)[:, :, :, ::-1]
        enc8i = np.ascontiguousarray(
            e8rev.reshape(NT, P, KC // 2, 2, P).transpose(0, 1, 2, 4, 3)
        ).reshape(NT, P, KC // 2, 2 * P)
        m = {
            "encv": np.ascontiguousarray(encv),
            "encT": encT,
            "encT8": enc8i,
            "wb": wb,
            "w8": w8,
        }
        if with_bias:
            m["bb"] = bb
            m["b8"] = b8
        in_maps.append(m)
    return in_maps, with_bias


def kernel(hidden_dec, hidden_enc, W, b):
    in_maps, with_bias = make_in_maps(hidden_dec, hidden_enc, W, b)
    nc = build_program(with_bias)
    res = bass_utils.run_bass_kernel_spmd(nc, in_maps, core_ids=list(range(NCORES)))
    outp = np.concatenate([res.results[c]["out"] for c in range(NCORES)], axis=0)
    outp[:, NB:] /= WT8SCALE  # undo the fp8 wt pre-scale on the low block
    perm = _perm(np.asarray(hidden_dec, dtype=np.float32).reshape(D))
    out = np.empty_like(outp)
    out[:, perm] = outp
    return out.astype(np.float32)
